# revision 13
# baseline (speedup 1.0000x reference)
# Trainium2 Bass kernel for single-head attention:
#   out = softmax((q@Wq+bq)(k@Wk+bk)^T / sqrt(D)) @ (v@Wv+bv) @ Wo + bo
# Full shapes: query/key/value [4, 2048, 1024], D=1024, mask all-ones.
#
# Sharding: data-parallel over (batch, query-half) -> 8 shards, one per
# NeuronCore. Core c handles batch b=c//2, query rows [h*1024, (h+1)*1024)
# with h=c%2. Each core projects only its OWN half of the batch's key/value
# tokens. The k-axis is PERMUTED per core: own tokens occupy k-positions
# [0, 1024) ("half A"), partner tokens [1024, 2048) ("half B") — softmax and
# P@V are permutation-invariant as long as K and V use the same order, so
# the SPMD program stays rank-independent. The partner half is obtained
# with a pairwise AllReduce(add) of the bf16 halves plus an on-chip
# subtract (partner = sum - own), which keeps every access pattern static.
#
# Per-core layout strategy: everything feature-major ("transposed") so the
# TensorEngine contracts over the partition dim with zero on-chip input
# transposes. Host pre-transposes inputs (free) and pre-casts to bf16.
#   qT/kT/vT  [D, 1024] (host-transposed shard, bf16)
#   KT = (Wk^T kT)+bk   [D, 2048]  via matmul(lhsT=Wk, rhs=kT chunks)
#   QT = (Wq^T qT)/32+bq [D, 1024]
#   V  token-major      [2048, D]  via matmul(lhsT=vT tile, rhs=Wv chunk)
#   scores[q,k] accumulates over d: matmul(lhsT=QT tile, rhs=KT chunk)
#   softmax: one ACT Exp pass per [128,1024] PSUM half (no max-subtraction:
#   |scores| <~ 8 for this distribution, exp is safe in fp32) with
#   accum_out giving the row-sum. P stays UNNORMALIZED; 1/rowsum commutes
#   through P@V and @Wo and is applied as a per-partition ACT scale in the
#   output projection, where query tokens are back on partitions.
#   P^T tiles via PE transpose; attn_outT[dv,q] = matmul(lhsT=V, rhs=P^T)
#   out[tok,dout] = matmul(lhsT=attn_outT tile, rhs=Wo chunk)
# The attention loop is software-pipelined: the partner-independent half-A
# work (scores+exp+transpose) of the first LOOKAHEAD q-tiles is emitted
# before any half-B work, giving the collective ~20us to land.
# Biases bv/bo are folded into a host-side additive constant
# bo' = bv @ Wo + bo (softmax rows sum to 1), added after the gather.

import functools

import ml_dtypes
import numpy as np

B, S, D = 4, 2048, 1024
N_CORES = 8
P = 128
DT = D // P        # 8 d-tiles of 128
TQ = S // 2        # 1024 query rows / kv-half rows per core
NQ = TQ // P       # 8 q-tiles
NK = S // P        # 16 k-tiles
NKH = NK // 2      # 8 k-tiles per half
SCALE = 1.0 / np.sqrt(np.float32(D))  # 1/32
BF16 = ml_dtypes.bfloat16
PAIRS = [[0, 1], [2, 3], [4, 5], [6, 7]]
LOOKAHEAD = 5      # q-tiles of half-A work emitted before half-B starts


@functools.lru_cache(maxsize=1)
def _build():
    import concourse.bass as bass  # noqa: F401  (registers engines)
    import concourse.mybir as mybir
    import concourse.tile as tile
    from concourse import bacc
    from concourse.masks import make_identity

    f32 = mybir.dt.float32
    bf16 = mybir.dt.bfloat16

    nc = bacc.Bacc("TRN2", target_bir_lowering=False, debug=False,
                   num_devices=N_CORES)

    qT = nc.dram_tensor("qT", [D, TQ], bf16, kind="ExternalInput")
    kT = nc.dram_tensor("kT", [D, TQ], bf16, kind="ExternalInput")
    vT = nc.dram_tensor("vT", [D, TQ], bf16, kind="ExternalInput")
    wq = nc.dram_tensor("wq", [D, D], bf16, kind="ExternalInput")
    wk = nc.dram_tensor("wk", [D, D], bf16, kind="ExternalInput")
    wv = nc.dram_tensor("wv", [D, D], bf16, kind="ExternalInput")
    wo = nc.dram_tensor("wo", [D, D], bf16, kind="ExternalInput")
    bq32 = nc.dram_tensor("bq32", [D], f32, kind="ExternalInput")  # bq/32
    bk_d = nc.dram_tensor("bk", [D], f32, kind="ExternalInput")
    out_d = nc.dram_tensor("out", [TQ, D], f32, kind="ExternalOutput")

    Ident = mybir.ActivationFunctionType.Identity
    Exp = mybir.ActivationFunctionType.Exp

    with tile.TileContext(nc) as tc:
        with (
            tc.tile_pool(name="const", bufs=1) as const,
            tc.tile_pool(name="wpool", bufs=2) as wpool,
            tc.tile_pool(name="xin", bufs=3) as xin,
            tc.tile_pool(name="big", bufs=1) as big,
            tc.tile_pool(name="work", bufs=2) as work,
            tc.tile_pool(name="phalf", bufs=3) as phalf,
            tc.tile_pool(name="ptp", bufs=NQ) as ptp,
            tc.tile_pool(name="ssums", bufs=NQ) as ssums,
            tc.tile_pool(name="dram", bufs=1, space="DRAM") as dram,
            tc.tile_pool(name="mmps", bufs=2, space="PSUM") as mmps,
            tc.tile_pool(name="scps", bufs=2, space="PSUM") as scps,
            tc.tile_pool(name="tpps", bufs=2, space="PSUM") as tpps,
        ):
            # ---- constants (gpsimd DMA queue; sync stays free for inputs) --
            bq_sb = const.tile([P, DT], f32, tag="bq")
            bk_sb = const.tile([P, DT], f32, tag="bk")
            nc.gpsimd.dma_start(bq_sb[:], bq32.ap().rearrange("(o p) -> p o", p=P))
            nc.gpsimd.dma_start(bk_sb[:], bk_d.ap().rearrange("(o p) -> p o", p=P))
            ident = const.tile([P, P], bf16, tag="ident")
            make_identity(nc, ident[:])
            r_all = const.tile([P, NQ], f32, tag="rall")

            # ---- persistent intermediates ----
            QT = big.tile([P, DT, TQ], bf16, tag="QT")       # 2 MB
            KT = big.tile([P, DT, S], bf16, tag="KT")        # 4 MB
            Vtm = big.tile([P, NK, D], bf16, tag="Vtm")      # 4 MB (token-major)
            aoT = big.tile([P, DT, TQ], bf16, tag="aoT")     # 2 MB attn_out^T

            # ---- weights (2 live at a time, on the gpsimd DMA queue) ----
            def load_w(dram_t):
                w = wpool.tile([P, DT, D], bf16, tag="w")
                ap = dram_t.ap().rearrange("(dt p) n -> p dt n", p=P)
                nc.gpsimd.dma_start(w[:, :, :D // 2], ap[:, :, :D // 2])
                nc.gpsimd.dma_start(w[:, :, D // 2:], ap[:, :, D // 2:])
                return w

            # out[:, o, tokc] = sum_dt W[:, dt, o*P:+P]^T @ xT[:, dt, tokc]
            def proj_fm(w_sb, x_dram, out_view, bias_col, scale):
                x_ap = x_dram.ap().rearrange("(dt p) t -> p dt t", p=P)
                for c in range(TQ // 512):
                    xt = xin.tile([P, DT, 512], bf16, tag="xin")
                    nc.sync.dma_start(xt[:], x_ap[:, :, c * 512:(c + 1) * 512])
                    for o in range(DT):
                        ps = mmps.tile([P, 512], f32, tag="mm")
                        for dt_i in range(DT):
                            nc.tensor.matmul(
                                ps[:],
                                w_sb[:, dt_i, o * P:(o + 1) * P],
                                xt[:, dt_i, :],
                                start=(dt_i == 0),
                                stop=(dt_i == DT - 1),
                            )
                        nc.scalar.activation(
                            out_view[:, o, c * 512:(c + 1) * 512], ps[:],
                            Ident,
                            bias=(bias_col[:, o:o + 1] if bias_col is not None
                                  else 0.0),
                            scale=scale,
                        )

            # ---- K projection (own half -> KT[:, :, 0:TQ]) ----
            w_k = load_w(wk)
            proj_fm(w_k, kT, KT[:, :, 0:TQ], bk_sb, 1.0)
            ex_k_in = dram.tile([P, DT, TQ], bf16)
            ex_k_out = dram.tile([P, DT, TQ], bf16)
            nc.scalar.dma_start(ex_k_in[:], KT[:, :, 0:TQ])
            nc.gpsimd.collective_compute(
                "AllReduce", mybir.AluOpType.add, replica_groups=PAIRS,
                ins=[ex_k_in.opt()], outs=[ex_k_out.opt()],
            )
            # partner K = sum - own, recovered in 1MB chunks
            for c in range(2):
                sm = xin.tile([P, DT, 512], bf16, tag="xin", name=f"smk{c}")
                nc.scalar.dma_start(
                    sm[:], ex_k_out[:, :, c * 512:(c + 1) * 512])
                nc.vector.tensor_tensor(
                    KT[:, :, TQ + c * 512:TQ + (c + 1) * 512],
                    sm[:], KT[:, :, c * 512:(c + 1) * 512],
                    mybir.AluOpType.subtract,
                )

            # ---- V projection (own half, token-major -> Vtm[:, 0:8, :]) ----
            w_v = load_w(wv)
            v_ap = vT.ap().rearrange("(dt p) t -> p dt t", p=P)
            for c in range(TQ // 512):
                xt = xin.tile([P, DT, 512], bf16, tag="xin")
                nc.sync.dma_start(xt[:], v_ap[:, :, c * 512:(c + 1) * 512])
                for sub in range(4):            # 4 tok-tiles per chunk
                    tt = c * 4 + sub
                    for dc in range(2):         # dout chunks of 512
                        ps = mmps.tile([P, 512], f32, tag="mm")
                        for dt_i in range(DT):
                            nc.tensor.matmul(
                                ps[:],
                                xt[:, dt_i, sub * P:(sub + 1) * P],
                                w_v[:, dt_i, dc * 512:(dc + 1) * 512],
                                start=(dt_i == 0),
                                stop=(dt_i == DT - 1),
                            )
                        nc.scalar.copy(Vtm[:, tt, dc * 512:(dc + 1) * 512],
                                       ps[:])
            ex_v_in = dram.tile([P, NKH, D], bf16)
            ex_v_out = dram.tile([P, NKH, D], bf16)
            nc.scalar.dma_start(ex_v_in[:], Vtm[:, 0:NKH, :])
            nc.gpsimd.collective_compute(
                "AllReduce", mybir.AluOpType.add, replica_groups=PAIRS,
                ins=[ex_v_in.opt()], outs=[ex_v_out.opt()],
            )
            # partner V = sum - own, recovered in 1MB chunks
            for c in range(2):
                sm = xin.tile([P, 4, D], bf16, tag="xin", name=f"smv{c}")
                nc.scalar.dma_start(sm[:], ex_v_out[:, c * 4:(c + 1) * 4, :])
                nc.vector.tensor_tensor(
                    Vtm[:, NKH + c * 4:NKH + (c + 1) * 4, :],
                    sm[:], Vtm[:, c * 4:(c + 1) * 4, :],
                    mybir.AluOpType.subtract,
                )

            # ---- Q projection ----
            w_q = load_w(wq)
            proj_fm(w_q, qT, QT, bq_sb, float(SCALE))
            w_o = load_w(wo)

            # ---- attention, software-pipelined over q-tiles ----
            # half_a(qi): scores+exp+transpose on own k-half (collective-free)
            # half_b(qi): same on partner half, then attn_outT accumulation
            pT_tiles = {}
            ssum_tiles = {}

            def half_pass(qi, half):
                qsl = slice(qi * P, (qi + 1) * P)
                sc = scps.tile([P, 1024], f32, tag="sc")
                for kc in range(2):
                    for dt_i in range(DT):
                        nc.tensor.matmul(
                            sc[:, kc * 512:(kc + 1) * 512],
                            QT[:, dt_i, qsl],
                            KT[:, dt_i, half * 1024 + kc * 512:
                               half * 1024 + (kc + 1) * 512],
                            start=(dt_i == 0),
                            stop=(dt_i == DT - 1),
                        )
                ph = phalf.tile([P, 1024], bf16, tag="ph")
                nc.scalar.activation(
                    ph[:], sc[:], Exp,
                    accum_out=ssum_tiles[qi][:, half:half + 1])
                pT = pT_tiles[qi]
                for kt in range(NKH):
                    tp = tpps.tile([P, P], bf16, tag="tp")
                    nc.tensor.transpose(
                        tp[:], ph[:, kt * P:(kt + 1) * P], ident[:])
                    nc.vector.tensor_copy(pT[:, half * NKH + kt, :], tp[:])

            def p1(qi):
                pT_tiles[qi] = ptp.tile([P, NK, P], bf16, tag="pT", name=f"pT{qi}")
                ssum_tiles[qi] = ssums.tile([P, 2], f32, tag="ssum", name=f"ssum{qi}")
                half_pass(qi, 0)

            def p2(qi):
                half_pass(qi, 1)
                stot = work.tile([P, 1], f32, tag="stot")
                nc.vector.tensor_add(
                    stot[:], ssum_tiles[qi][:, 0:1], ssum_tiles[qi][:, 1:2])
                nc.vector.reciprocal(r_all[:, qi:qi + 1], stot[:])

            def p3(qi):
                # attn_outT accumulation (needs the full Vtm, incl. partner)
                qsl = slice(qi * P, (qi + 1) * P)
                pT = pT_tiles[qi]
                for dvt in range(DT):
                    av = mmps.tile([P, 512], f32, tag="mm")
                    for kt in range(NK):
                        nc.tensor.matmul(
                            av[:, :P],
                            Vtm[:, kt, dvt * P:(dvt + 1) * P],
                            pT[:, kt, :],
                            start=(kt == 0),
                            stop=(kt == NK - 1),
                        )
                    nc.vector.tensor_copy(aoT[:, dvt, qsl], av[:, :P])

            def out_proj(tt):
                # out[tok, dout], scaled by 1/rowsum (tokens on partitions)
                fin = work.tile([P, D], f32, tag="fin")
                for dc in range(2):
                    ps = mmps.tile([P, 512], f32, tag="mm")
                    for dvt in range(DT):
                        nc.tensor.matmul(
                            ps[:],
                            aoT[:, dvt, tt * P:(tt + 1) * P],
                            w_o[:, dvt, dc * 512:(dc + 1) * 512],
                            start=(dvt == 0),
                            stop=(dvt == DT - 1),
                        )
                    nc.scalar.activation(
                        fin[:, dc * 512:(dc + 1) * 512], ps[:],
                        Ident, scale=r_all[:, tt:tt + 1],
                    )
                nc.sync.dma_start(out_d.ap()[tt * P:(tt + 1) * P, :], fin[:])

            for qi in range(min(LOOKAHEAD, NQ)):
                p1(qi)
            for qi in range(NQ):
                p2(qi)
                if qi + LOOKAHEAD < NQ:
                    p1(qi + LOOKAHEAD)
            for qi in range(NQ):
                p3(qi)
                out_proj(qi)

    nc.compile()
    return nc


def _numpy_reference(query, key, value, mask, Wq, bq, Wk, bk, Wv, bv, Wo, bo):
    q = query @ Wq + bq
    k = key @ Wk + bk
    v = value @ Wv + bv
    s = np.einsum("bsd,btd->bst", q, k) / np.sqrt(np.float32(q.shape[-1]))
    s = np.where(mask == 0, np.float32(-1e9), s)
    s = s - s.max(axis=-1, keepdims=True)
    e = np.exp(s)
    p = e / e.sum(axis=-1, keepdims=True)
    x = np.einsum("bst,btd->bsd", p, v)
    return (x @ Wo + bo).astype(np.float32)


def kernel(query, key, value, mask, Wq, bq, Wk, bk, Wv, bv, Wo, bo):
    query = np.asarray(query, np.float32)
    key = np.asarray(key, np.float32)
    value = np.asarray(value, np.float32)
    mask = np.asarray(mask)
    if not np.all(mask != 0):
        # This problem's mask is always all-ones; keep a correct fallback.
        return _numpy_reference(query, key, value, mask, Wq, bq, Wk, bk,
                                Wv, bv, Wo, bo)

    from concourse.bass_utils import run_bass_kernel_spmd

    nc = _build()

    wq_b = np.ascontiguousarray(np.asarray(Wq, np.float32)).astype(BF16)
    wk_b = np.ascontiguousarray(np.asarray(Wk, np.float32)).astype(BF16)
    wv_b = np.ascontiguousarray(np.asarray(Wv, np.float32)).astype(BF16)
    wo_b = np.ascontiguousarray(np.asarray(Wo, np.float32)).astype(BF16)
    bq32 = (np.asarray(bq, np.float32) * SCALE).astype(np.float32)
    bk_f = np.asarray(bk, np.float32)
    bo_eff = (np.asarray(bv, np.float32) @ np.asarray(Wo, np.float32)
              + np.asarray(bo, np.float32)).astype(np.float32)

    in_maps = []
    for c in range(N_CORES):
        b, h = divmod(c, 2)
        sl = slice(h * TQ, (h + 1) * TQ)
        in_maps.append({
            "qT": np.ascontiguousarray(query[b, sl].T).astype(BF16),
            "kT": np.ascontiguousarray(key[b, sl].T).astype(BF16),
            "vT": np.ascontiguousarray(value[b, sl].T).astype(BF16),
            "wq": wq_b, "wk": wk_b, "wv": wv_b, "wo": wo_b,
            "bq32": bq32, "bk": bk_f,
        })

    global _last_in_maps
    _last_in_maps = in_maps
    res = run_bass_kernel_spmd(nc, in_maps, list(range(N_CORES)))

    out = np.empty((B, S, D), np.float32)
    for c in range(N_CORES):
        b, h = divmod(c, 2)
        out[b, h * TQ:(h + 1) * TQ] = res.results[c]["out"]
    out += bo_eff
    return out


# revision 14
# speedup vs baseline: 1.2408x; 1.2408x over previous
# Trainium2 Bass kernel for single-head attention:
#   out = softmax((q@Wq+bq)(k@Wk+bk)^T / sqrt(D)) @ (v@Wv+bv) @ Wo + bo
# Full shapes: query/key/value [4, 2048, 1024], D=1024, mask all-ones.
#
# Sharding: data-parallel over (batch, query-half) -> 8 shards, one per
# NeuronCore. Core c handles batch b=c//2, query rows [h*1024, (h+1)*1024)
# with h=c%2. Each core projects only its OWN half of the batch's key/value
# tokens. The k-axis is PERMUTED per core: own tokens occupy k-positions
# [0, 1024) ("half A"), partner tokens [1024, 2048) ("half B") — softmax and
# P@V are permutation-invariant as long as K and V use the same order, so
# the SPMD program stays rank-independent. The partner half is obtained
# with a pairwise AllReduce(add) of the bf16 halves plus an on-chip
# subtract (partner = sum - own), which keeps every access pattern static.
#
# Per-core layout strategy: everything feature-major ("transposed") so the
# TensorEngine contracts over the partition dim with zero on-chip input
# transposes. Host pre-transposes inputs (free) and pre-casts to bf16.
#   qT/kT/vT  [D, 1024] (host-transposed shard, bf16)
#   KT = (Wk^T kT)+bk   [D, 2048]  via matmul(lhsT=Wk, rhs=kT chunks)
#   QT = (Wq^T qT)/32+bq [D, 1024]
#   V  token-major      [2048, D]  via matmul(lhsT=vT tile, rhs=Wv chunk)
#   scores[q,k] accumulates over d: matmul(lhsT=QT tile, rhs=KT chunk)
#   softmax: one ACT Exp pass per [128,1024] PSUM half (no max-subtraction:
#   |scores| <~ 8 for this distribution, exp is safe in fp32) with
#   accum_out giving the row-sum. P stays UNNORMALIZED; 1/rowsum commutes
#   through P@V and @Wo and is applied as a per-partition ACT scale in the
#   output projection, where query tokens are back on partitions.
#   P^T tiles via PE transpose; attn_outT[dv,q] = matmul(lhsT=V, rhs=P^T)
#   out[tok,dout] = matmul(lhsT=attn_outT tile, rhs=Wo chunk)
# The attention loop is software-pipelined: the partner-independent half-A
# work (scores+exp+transpose) of the first LOOKAHEAD q-tiles is emitted
# before any half-B work, giving the collective ~20us to land.
# Biases bv/bo are folded into a host-side additive constant
# bo' = bv @ Wo + bo (softmax rows sum to 1), added after the gather.

import functools

import ml_dtypes
import numpy as np

B, S, D = 4, 2048, 1024
N_CORES = 8
P = 128
DT = D // P        # 8 d-tiles of 128
TQ = S // 2        # 1024 query rows / kv-half rows per core
NQ = TQ // P       # 8 q-tiles
NK = S // P        # 16 k-tiles
NKH = NK // 2      # 8 k-tiles per half
SCALE = 1.0 / np.sqrt(np.float32(D))  # 1/32
BF16 = ml_dtypes.bfloat16
PAIRS = [[0, 1], [2, 3], [4, 5], [6, 7]]
LOOKAHEAD = 5      # q-tiles of half-A work emitted before half-B starts


@functools.lru_cache(maxsize=1)
def _build():
    import concourse.bass as bass  # noqa: F401  (registers engines)
    import concourse.mybir as mybir
    import concourse.tile as tile
    from concourse import bacc
    from concourse.masks import make_identity

    f32 = mybir.dt.float32
    bf16 = mybir.dt.bfloat16

    nc = bacc.Bacc("TRN2", target_bir_lowering=False, debug=False,
                   num_devices=N_CORES)

    qT = nc.dram_tensor("qT", [D, TQ], bf16, kind="ExternalInput")
    kT = nc.dram_tensor("kT", [D, TQ], bf16, kind="ExternalInput")
    vT = nc.dram_tensor("vT", [D, TQ], bf16, kind="ExternalInput")
    wq = nc.dram_tensor("wq", [D, D], bf16, kind="ExternalInput")
    wk = nc.dram_tensor("wk", [D, D], bf16, kind="ExternalInput")
    wv = nc.dram_tensor("wv", [D, D], bf16, kind="ExternalInput")
    wo = nc.dram_tensor("wo", [D, D], bf16, kind="ExternalInput")
    bq32 = nc.dram_tensor("bq32", [D], f32, kind="ExternalInput")  # bq/32
    bk_d = nc.dram_tensor("bk", [D], f32, kind="ExternalInput")
    out_d = nc.dram_tensor("out", [TQ, D], f32, kind="ExternalOutput")

    Ident = mybir.ActivationFunctionType.Identity
    Exp = mybir.ActivationFunctionType.Exp

    with tile.TileContext(nc) as tc:
        with (
            tc.tile_pool(name="const", bufs=1) as const,
            tc.tile_pool(name="wpool", bufs=2) as wpool,
            tc.tile_pool(name="xin", bufs=2) as xin,
            tc.tile_pool(name="smkv", bufs=1) as smkv,
            tc.tile_pool(name="big", bufs=1) as big,
            tc.tile_pool(name="work", bufs=2) as work,
            tc.tile_pool(name="phalf", bufs=2) as phalf,
            tc.tile_pool(name="ptp", bufs=NQ) as ptp,
            tc.tile_pool(name="ssums", bufs=NQ) as ssums,
            tc.tile_pool(name="dram", bufs=1, space="DRAM") as dram,
            tc.tile_pool(name="mmps", bufs=2, space="PSUM") as mmps,
            tc.tile_pool(name="scps", bufs=2, space="PSUM") as scps,
            tc.tile_pool(name="tpps", bufs=2, space="PSUM") as tpps,
        ):
            # ---- constants (gpsimd DMA queue; sync stays free for inputs) --
            bq_sb = const.tile([P, DT], f32, tag="bq")
            bk_sb = const.tile([P, DT], f32, tag="bk")
            nc.gpsimd.dma_start(bq_sb[:], bq32.ap().rearrange("(o p) -> p o", p=P))
            nc.gpsimd.dma_start(bk_sb[:], bk_d.ap().rearrange("(o p) -> p o", p=P))
            ident = const.tile([P, P], bf16, tag="ident")
            make_identity(nc, ident[:])
            r_all = const.tile([P, NQ], f32, tag="rall")

            # ---- persistent intermediates ----
            QT = big.tile([P, DT, TQ], bf16, tag="QT")       # 2 MB
            KT = big.tile([P, DT, S], bf16, tag="KT")        # 4 MB
            Vtm = big.tile([P, NK, D], bf16, tag="Vtm")      # 4 MB (token-major)
            aoT = big.tile([P, DT, TQ], bf16, tag="aoT")     # 2 MB attn_out^T

            # ---- weights (2 live at a time, on the gpsimd DMA queue) ----
            def load_w(dram_t):
                w = wpool.tile([P, DT, D], bf16, tag="w")
                ap = dram_t.ap().rearrange("(dt p) n -> p dt n", p=P)
                nc.gpsimd.dma_start(w[:, :, :D // 2], ap[:, :, :D // 2])
                nc.gpsimd.dma_start(w[:, :, D // 2:], ap[:, :, D // 2:])
                return w

            # out[:, o, tokc] = sum_dt W[:, dt, o*P:+P]^T @ xT[:, dt, tokc]
            def proj_fm(w_sb, x_dram, out_view, bias_col, scale):
                x_ap = x_dram.ap().rearrange("(dt p) t -> p dt t", p=P)
                for c in range(TQ // 512):
                    xt = xin.tile([P, DT, 512], bf16, tag="xin")
                    nc.sync.dma_start(xt[:], x_ap[:, :, c * 512:(c + 1) * 512])
                    for o in range(DT):
                        ps = mmps.tile([P, 512], f32, tag="mm")
                        for dt_i in range(DT):
                            nc.tensor.matmul(
                                ps[:],
                                w_sb[:, dt_i, o * P:(o + 1) * P],
                                xt[:, dt_i, :],
                                start=(dt_i == 0),
                                stop=(dt_i == DT - 1),
                            )
                        nc.scalar.activation(
                            out_view[:, o, c * 512:(c + 1) * 512], ps[:],
                            Ident,
                            bias=(bias_col[:, o:o + 1] if bias_col is not None
                                  else 0.0),
                            scale=scale,
                        )

            # ---- K projection (own half -> KT[:, :, 0:TQ]) ----
            # Weight loads and the partner-recovery DMAs all live on the
            # gpsimd queue, AFTER the collective triggers, so no compute
            # engine's queue ever blocks behind a collective wait.
            w_k = load_w(wk)
            w_v = load_w(wv)
            proj_fm(w_k, kT, KT[:, :, 0:TQ], bk_sb, 1.0)
            ex_k_in = dram.tile([P, DT, TQ], bf16)
            ex_k_out = dram.tile([P, DT, TQ], bf16)
            nc.scalar.dma_start(ex_k_in[:], KT[:, :, 0:TQ])
            nc.gpsimd.collective_compute(
                "AllReduce", mybir.AluOpType.add, replica_groups=PAIRS,
                ins=[ex_k_in.opt()], outs=[ex_k_out.opt()],
            )
            w_q = load_w(wq)

            # ---- V projection (own half, token-major -> Vtm[:, 0:8, :]) ----
            v_ap = vT.ap().rearrange("(dt p) t -> p dt t", p=P)
            for c in range(TQ // 512):
                xt = xin.tile([P, DT, 512], bf16, tag="xin")
                nc.sync.dma_start(xt[:], v_ap[:, :, c * 512:(c + 1) * 512])
                for sub in range(4):            # 4 tok-tiles per chunk
                    tt = c * 4 + sub
                    for dc in range(2):         # dout chunks of 512
                        ps = mmps.tile([P, 512], f32, tag="mm")
                        for dt_i in range(DT):
                            nc.tensor.matmul(
                                ps[:],
                                xt[:, dt_i, sub * P:(sub + 1) * P],
                                w_v[:, dt_i, dc * 512:(dc + 1) * 512],
                                start=(dt_i == 0),
                                stop=(dt_i == DT - 1),
                            )
                        nc.scalar.copy(Vtm[:, tt, dc * 512:(dc + 1) * 512],
                                       ps[:])
            ex_v_in = dram.tile([P, NKH, D], bf16)
            ex_v_out = dram.tile([P, NKH, D], bf16)
            nc.scalar.dma_start(ex_v_in[:], Vtm[:, 0:NKH, :])
            nc.gpsimd.collective_compute(
                "AllReduce", mybir.AluOpType.add, replica_groups=PAIRS,
                ins=[ex_v_in.opt()], outs=[ex_v_out.opt()],
            )
            w_o = load_w(wo)

            # recovery staging DMAs (gpsimd queue tail; they wait on the
            # collectives without blocking anything else)
            sm_k, sm_v = [], []
            for c in range(2):
                sm = smkv.tile([P, DT, 512], bf16, tag="sm", name=f"smk{c}")
                nc.gpsimd.dma_start(
                    sm[:], ex_k_out[:, :, c * 512:(c + 1) * 512])
                sm_k.append(sm)
            for c in range(2):
                sm = smkv.tile([P, 4, D], bf16, tag="sm", name=f"smv{c}")
                nc.gpsimd.dma_start(sm[:], ex_v_out[:, c * 4:(c + 1) * 4, :])
                sm_v.append(sm)

            def sub_k(c):   # partner K = sum - own (DVE)
                nc.vector.tensor_tensor(
                    KT[:, :, TQ + c * 512:TQ + (c + 1) * 512],
                    sm_k[c][:], KT[:, :, c * 512:(c + 1) * 512],
                    mybir.AluOpType.subtract,
                )

            def sub_v(c):   # partner V = sum - own (DVE)
                nc.vector.tensor_tensor(
                    Vtm[:, NKH + c * 4:NKH + (c + 1) * 4, :],
                    sm_v[c][:], Vtm[:, c * 4:(c + 1) * 4, :],
                    mybir.AluOpType.subtract,
                )

            # ---- Q projection ----
            proj_fm(w_q, qT, QT, bq_sb, float(SCALE))

            # ---- attention, software-pipelined over q-tiles ----
            # half_a(qi): scores+exp+transpose on own k-half (collective-free)
            # half_b(qi): same on partner half, then attn_outT accumulation
            pT_tiles = {}
            ssum_tiles = {}

            def half_pass(qi, half):
                qsl = slice(qi * P, (qi + 1) * P)
                sc = scps.tile([P, 1024], f32, tag="sc")
                for kc in range(2):
                    for dt_i in range(DT):
                        nc.tensor.matmul(
                            sc[:, kc * 512:(kc + 1) * 512],
                            QT[:, dt_i, qsl],
                            KT[:, dt_i, half * 1024 + kc * 512:
                               half * 1024 + (kc + 1) * 512],
                            start=(dt_i == 0),
                            stop=(dt_i == DT - 1),
                        )
                ph = phalf.tile([P, 1024], bf16, tag="ph")
                nc.scalar.activation(
                    ph[:], sc[:], Exp,
                    accum_out=ssum_tiles[qi][:, half:half + 1])
                pT = pT_tiles[qi]
                for kt in range(NKH):
                    tp = tpps.tile([P, P], bf16, tag="tp")
                    nc.tensor.transpose(
                        tp[:], ph[:, kt * P:(kt + 1) * P], ident[:])
                    nc.vector.tensor_copy(pT[:, half * NKH + kt, :], tp[:])

            def p1(qi):
                pT_tiles[qi] = ptp.tile([P, NK, P], bf16, tag="pT", name=f"pT{qi}")
                ssum_tiles[qi] = ssums.tile([P, 2], f32, tag="ssum", name=f"ssum{qi}")
                half_pass(qi, 0)

            def p2(qi):
                half_pass(qi, 1)
                stot = work.tile([P, 1], f32, tag="stot")
                nc.vector.tensor_add(
                    stot[:], ssum_tiles[qi][:, 0:1], ssum_tiles[qi][:, 1:2])
                nc.vector.reciprocal(r_all[:, qi:qi + 1], stot[:])

            def p3(qi):
                # attn_outT accumulation (needs the full Vtm, incl. partner)
                qsl = slice(qi * P, (qi + 1) * P)
                pT = pT_tiles[qi]
                for dvt in range(DT):
                    av = mmps.tile([P, 512], f32, tag="mm")
                    for kt in range(NK):
                        nc.tensor.matmul(
                            av[:, :P],
                            Vtm[:, kt, dvt * P:(dvt + 1) * P],
                            pT[:, kt, :],
                            start=(kt == 0),
                            stop=(kt == NK - 1),
                        )
                    nc.vector.tensor_copy(aoT[:, dvt, qsl], av[:, :P])

            def out_proj(tt):
                # out[tok, dout], scaled by 1/rowsum (tokens on partitions)
                fin = work.tile([P, D], f32, tag="fin")
                for dc in range(2):
                    ps = mmps.tile([P, 512], f32, tag="mm")
                    for dvt in range(DT):
                        nc.tensor.matmul(
                            ps[:],
                            aoT[:, dvt, tt * P:(tt + 1) * P],
                            w_o[:, dvt, dc * 512:(dc + 1) * 512],
                            start=(dvt == 0),
                            stop=(dvt == DT - 1),
                        )
                    nc.scalar.activation(
                        fin[:, dc * 512:(dc + 1) * 512], ps[:],
                        Ident, scale=r_all[:, tt:tt + 1],
                    )
                nc.sync.dma_start(out_d.ap()[tt * P:(tt + 1) * P, :], fin[:])

            for qi in range(min(LOOKAHEAD, NQ)):
                p1(qi)
                if qi == 0:
                    sub_k(0)
                    sub_k(1)
            for qi in range(NQ):
                p2(qi)
                if qi == 2:
                    sub_v(0)
                elif qi == 4:
                    sub_v(1)
                if qi + LOOKAHEAD < NQ:
                    p1(qi + LOOKAHEAD)
            for qi in range(NQ):
                p3(qi)
                out_proj(qi)

    nc.compile()
    return nc


def _numpy_reference(query, key, value, mask, Wq, bq, Wk, bk, Wv, bv, Wo, bo):
    q = query @ Wq + bq
    k = key @ Wk + bk
    v = value @ Wv + bv
    s = np.einsum("bsd,btd->bst", q, k) / np.sqrt(np.float32(q.shape[-1]))
    s = np.where(mask == 0, np.float32(-1e9), s)
    s = s - s.max(axis=-1, keepdims=True)
    e = np.exp(s)
    p = e / e.sum(axis=-1, keepdims=True)
    x = np.einsum("bst,btd->bsd", p, v)
    return (x @ Wo + bo).astype(np.float32)


def kernel(query, key, value, mask, Wq, bq, Wk, bk, Wv, bv, Wo, bo):
    query = np.asarray(query, np.float32)
    key = np.asarray(key, np.float32)
    value = np.asarray(value, np.float32)
    mask = np.asarray(mask)
    if not np.all(mask != 0):
        # This problem's mask is always all-ones; keep a correct fallback.
        return _numpy_reference(query, key, value, mask, Wq, bq, Wk, bk,
                                Wv, bv, Wo, bo)

    from concourse.bass_utils import run_bass_kernel_spmd

    nc = _build()

    wq_b = np.ascontiguousarray(np.asarray(Wq, np.float32)).astype(BF16)
    wk_b = np.ascontiguousarray(np.asarray(Wk, np.float32)).astype(BF16)
    wv_b = np.ascontiguousarray(np.asarray(Wv, np.float32)).astype(BF16)
    wo_b = np.ascontiguousarray(np.asarray(Wo, np.float32)).astype(BF16)
    bq32 = (np.asarray(bq, np.float32) * SCALE).astype(np.float32)
    bk_f = np.asarray(bk, np.float32)
    bo_eff = (np.asarray(bv, np.float32) @ np.asarray(Wo, np.float32)
              + np.asarray(bo, np.float32)).astype(np.float32)

    in_maps = []
    for c in range(N_CORES):
        b, h = divmod(c, 2)
        sl = slice(h * TQ, (h + 1) * TQ)
        in_maps.append({
            "qT": np.ascontiguousarray(query[b, sl].T).astype(BF16),
            "kT": np.ascontiguousarray(key[b, sl].T).astype(BF16),
            "vT": np.ascontiguousarray(value[b, sl].T).astype(BF16),
            "wq": wq_b, "wk": wk_b, "wv": wv_b, "wo": wo_b,
            "bq32": bq32, "bk": bk_f,
        })

    global _last_in_maps
    _last_in_maps = in_maps
    res = run_bass_kernel_spmd(nc, in_maps, list(range(N_CORES)))

    out = np.empty((B, S, D), np.float32)
    for c in range(N_CORES):
        b, h = divmod(c, 2)
        out[b, h * TQ:(h + 1) * TQ] = res.results[c]["out"]
    out += bo_eff
    return out
